# revision 39
# baseline (speedup 1.0000x reference)
"""BitNet attention SPMD kernel for 8 Trainium2 NeuronCores.

Problem: nn_BitNetAttention (B=2, N=2048, C=768, H=12, D=64).

Sharding: data-parallel over batch (2 groups of 4 cores); within a group each
core owns 512 query tokens.  Each core recomputes the full K/V for its batch
(collective-free), runs its N/4 x N attention slab for all 12 heads, and
produces its own [512, 768] slice of the final output.  The host concatenates
the 8 slices.

Numerics:
- BitNet quantized matmuls (qkv, proj) run as exact integer arithmetic: int8
  activations and ternary weights are exactly representable in bf16, and fp32
  PSUM accumulation of <=2^24 magnitudes is exact.  Dequantization scales are
  folded into cheap per-token column/broadcast multiplies.
- Attention matmuls (QK^T, AV) run in float32r (full PE rate, ~1.6e-4 rel).
- Softmax skips the max-subtraction (logits are O(1) by construction); the
  denominator comes from a ones-column appended to V and is divided out after
  a small per-head PE transpose.
"""
import sys
sys.path.insert(0, "/opt/trn_rl_repo")

import numpy as np
from contextlib import ExitStack

import concourse.bass as bass
import concourse.mybir as mybir
import concourse.tile as tile
import concourse.bacc as bacc
from concourse.bass_utils import run_bass_kernel_spmd

dt = mybir.dt
AF = mybir.ActivationFunctionType
ALU = mybir.AluOpType
AX = mybir.AxisListType

B, N, C = 2, 2048, 768
H, D = 12, 64
NQ = N // 4              # 512 query tokens per core
TKV = N // 128           # 16 kv token chunks
TQ = NQ // 128           # 4 q token chunks
NCC = C // 128           # 6 contraction chunks
EPS = 1e-5
MAGIC = 12582912.0       # 1.5*2^23: x+MAGIC lands in [2^23,2^24) where ulp=1

_CACHE = {}


QUAKE = 0x5F3759DF


def _rsqrt_col(nc, st, ms):
    """rstd = 1/sqrt(ms) on DVE via the bit-trick seed + 2 Newton steps.

    Error after two steps is ~3e-11 rel — indistinguishable from a rounded
    fp32 rsqrt.  Avoids the ACT Sqrt table set (keeps ACT exp-only, so the
    program pays a single act-table load).
    """
    ihalf = st.tile([128, 1], dt.int32, tag="ihalf")
    nc.vector.tensor_scalar(ihalf[:], ms.bitcast(dt.int32), 1, None,
                            op0=ALU.arith_shift_right)
    y0 = st.tile([128, 1], dt.float32, tag="y0")
    nc.vector.tensor_scalar(y0[:].bitcast(dt.int32), ihalf[:], -1, QUAKE,
                            op0=ALU.mult, op1=ALU.add)
    y = y0
    for it in range(3):
        t1 = st.tile([128, 1], dt.float32, tag=f"nw{it}a")
        nc.vector.tensor_tensor(t1[:], y[:], y[:], op=ALU.mult)
        t2 = st.tile([128, 1], dt.float32, tag=f"nw{it}b")
        nc.vector.tensor_tensor(t2[:], t1[:], ms, op=ALU.mult)
        t3 = st.tile([128, 1], dt.float32, tag=f"nw{it}c")
        nc.vector.tensor_scalar(t3[:], t2[:], -0.5, 1.5, op0=ALU.mult,
                                op1=ALU.add)
        y1 = st.tile([128, 1], dt.float32, tag=f"nw{it}d")
        nc.vector.tensor_tensor(y1[:], t3[:], y[:], op=ALU.mult)
        y = y1
    return y


def _quant_x_tile(nc, pools, x_t, g_bc, inv_s_dst):
    """RMSNorm + per-token int8 absmax quant of one [128, C] tile.

    Returns xq (bf16, integer-valued, [128, C]).  Writes the inverse scale
    column (= clip(amax,eps)/127) into inv_s_dst ([128,1] AP).
    sum(x^2) runs on GPSIMD, the normalize/round chain on DVE, and the final
    magic-constant subtraction on ACT — ACT itself stays exp-table-only.
    """
    sc, st = pools["scratch"], pools["stats"]
    xsq = pools.get("dump", sc).tile([128, C], dt.float32, tag="xsq")
    sumsq = st.tile([128, 1], dt.float32, tag="sumsq")
    nc.scalar.activation(xsq[:], x_t, AF.Square, accum_out=sumsq[:])
    ms = st.tile([128, 1], dt.float32, tag="ms")
    nc.vector.tensor_scalar(ms[:], sumsq[:], float(1.0 / C), EPS,
                            op0=ALU.mult, op1=ALU.add)
    rstd = _rsqrt_col(nc, st, ms[:])
    xn = sc.tile([128, C], dt.float32, tag="xn")
    nc.vector.scalar_tensor_tensor(xn[:], x_t, rstd[:], g_bc[:],
                                   op0=ALU.mult, op1=ALU.mult)
    amax = st.tile([128, 1], dt.float32, tag="amax")
    nc.vector.tensor_reduce(amax[:], xn[:], axis=AX.X, op=ALU.max,
                            apply_absolute_value=True)
    amax_c = st.tile([128, 1], dt.float32, tag="amax_c")
    nc.vector.tensor_scalar(amax_c[:], amax[:], EPS, None, op0=ALU.max)
    r_amax = st.tile([128, 1], dt.float32, tag="r_amax")
    nc.vector.reciprocal(r_amax[:], amax_c[:])
    s_col = st.tile([128, 1], dt.float32, tag="s_col")
    nc.vector.tensor_scalar(s_col[:], r_amax[:], 127.0, None, op0=ALU.mult)
    nc.vector.tensor_scalar(inv_s_dst, amax_c[:], float(1.0 / 127.0), None,
                            op0=ALU.mult)
    t_r = sc.tile([128, C], dt.float32, tag="t_r")
    nc.vector.tensor_scalar(t_r[:], xn[:], s_col[:], MAGIC,
                            op0=ALU.mult, op1=ALU.add)
    xq = pools["xq"].tile([128, C], dt.bfloat16, tag="xq")
    nc.scalar.activation(xq[:], t_r[:], AF.Copy, bias=-MAGIC, scale=1.0)
    return xq


def build_program(debug_taps=False):
    nc = bacc.Bacc("TRN2", target_bir_lowering=False, debug=False, num_devices=8)

    xb_d = nc.dram_tensor("xb", [N, C], dt.float32, kind="ExternalInput")
    xm_d = nc.dram_tensor("xm", [NQ, C], dt.float32, kind="ExternalInput")
    wq_d = nc.dram_tensor("wqkv", [3 * C, C], dt.float32, kind="ExternalInput")
    wp_d = nc.dram_tensor("wproj", [C, C], dt.float32, kind="ExternalInput")
    gq_d = nc.dram_tensor("gq", [1, C], dt.float32, kind="ExternalInput")
    gp_d = nc.dram_tensor("gp", [1, C], dt.float32, kind="ExternalInput")
    out_d = nc.dram_tensor("out", [NQ, C], dt.float32, kind="ExternalOutput")
    dbg = {}
    if debug_taps:
        for nm, shape, d in (
                ("xq0", [128, C], dt.bfloat16),
                ("inv_s_all", [128, TKV], dt.float32),
                ("inv_s_my", [128, TQ], dt.float32),
                ("wqkvT0", [128, 3 * C], dt.bfloat16),
                ("qt0", [128, NQ], dt.float32),
                ("kt0", [128, N], dt.float32),
                ("vt0", [128, H * (D + 1)], dt.float32),
                ("alpha", [128, NQ], dt.float32),
                ("avsb0", [D + 1, NQ], dt.float32),
                ("att0", [128, C], dt.float32)):
            dbg[nm] = nc.dram_tensor(f"dbg_{nm}", shape, d, kind="ExternalOutput")

    with tile.TileContext(nc) as tc, ExitStack() as ctx:
        # ---- persistent pools & constants --------------------------------
        const = ctx.enter_context(tc.tile_pool(name="const", bufs=1))
        stats = ctx.enter_context(tc.tile_pool(name="stats", bufs=6))
        wT = ctx.enter_context(tc.tile_pool(name="wT", bufs=1))
        attout_p = ctx.enter_context(tc.tile_pool(name="attout", bufs=1))

        warm = const.tile([1, 1], dt.float32)
        nc.vector.memset(warm[:], 0.0)
        warm2 = const.tile([1, 1], dt.float32)
        nc.scalar.activation(warm2[:], warm[:], AF.Square)  # act-table load @ t=0

        ones_row = const.tile([1, 128], dt.float32)
        nc.vector.memset(ones_row[:], 1.0)
        ones_col = const.tile([128, 1], dt.float32)
        nc.vector.memset(ones_col[:], 1.0)
        eps_col = const.tile([128, 1], dt.float32)
        nc.vector.memset(eps_col[:], EPS)

        iota_c = const.tile([128, 1], dt.int32)
        nc.gpsimd.iota(iota_c[:], pattern=[[0, 1]], channel_multiplier=1)
        iota_r = const.tile([128, 128], dt.int32)
        nc.gpsimd.iota(iota_r[:], pattern=[[1, 128]], channel_multiplier=0)
        iota_cf = const.tile([128, 1], dt.float32)
        nc.vector.tensor_copy(iota_cf[:], iota_c[:])
        iota_rf = const.tile([128, 128], dt.float32)
        nc.vector.tensor_copy(iota_rf[:], iota_r[:])
        ident = const.tile([128, 128], dt.float32)
        nc.vector.tensor_scalar(ident[:], iota_rf[:], iota_cf[:], None,
                                op0=ALU.is_equal)

        gq_bc = const.tile([128, C], dt.float32)
        gp_bc = const.tile([128, C], dt.float32)
        with tc.tile_pool(name="grow", bufs=1) as grow, \
             tc.tile_pool(name="bc_ps", bufs=2, space="PSUM") as bc_ps:
            gq_row = grow.tile([1, C], dt.float32)
            nc.sync.dma_start(gq_row[:], gq_d.ap())
            gp_row = grow.tile([1, C], dt.float32)
            nc.sync.dma_start(gp_row[:], gp_d.ap())
            for row, bc in ((gq_row, gq_bc), (gp_row, gp_bc)):
                for lo in (0, 512):
                    hi = min(lo + 512, C)
                    ps = bc_ps.tile([128, 512], dt.float32, tag="gbc")
                    nc.tensor.matmul(ps[:, 0:hi - lo], ones_row[:],
                                     row[:, lo:hi], start=True, stop=True)
                    nc.vector.tensor_copy(bc[:, lo:hi], ps[:, 0:hi - lo])

        inv_s_all = const.tile([128, TKV], dt.float32)
        inv_s_my = const.tile([128, TQ], dt.float32)

        # transposed quantized weights: wqkvT[p, cc, o] = wq_qkv[o, 128cc+p]
        wqkvT = wT.tile([128, NCC, 3 * C], dt.bfloat16)
        wprojT = wT.tile([128, NCC, C], dt.bfloat16)

        # ---- phase W: weight quantization --------------------------------
        # All input DMAs are emitted first (w then x) so no load ever queues
        # behind the quant-gated weight transposes on the DMA pipe.
        xstage = ctx.enter_context(tc.tile_pool(name="xstage", bufs=4))
        scales = {}  # wname -> (thr_col, nthr_col, inv_sw_col, meanc11)
        with tc.tile_pool(name="wf32", bufs=1) as wf32, \
             tc.tile_pool(name="wq_st", bufs=3) as wq_st, \
             tc.tile_pool(name="w_ps", bufs=2, space="PSUM") as w_ps:
            wtiles_all = {}
            for wname, w_d, n_big in (("q", wq_d, 3), ("p", wp_d, 1)):
                wtiles_all[wname] = []
                for g in range(n_big):
                    w_t = wf32.tile([128, NCC, C], dt.float32, name=f"w_{wname}{g}")
                    src = w_d.ap()[g * 768:(g + 1) * 768, :].rearrange(
                        "(s p) c -> p s c", p=128)
                    nc.sync.dma_start(w_t[:], src)
                    wtiles_all[wname].append(w_t)

            x_tiles = []
            for i in range(10):
                src_d, off = (xm_d, i * 256) if i < 2 else (xb_d, (i - 2) * 256)
                xt = xstage.tile([128, 2, C], dt.float32, tag="xbig",
                                 name=f"xbig{i}")
                nc.sync.dma_start(
                    xt[:], src_d.ap()[off:off + 256, :].rearrange(
                        "(s p) c -> p s c", p=128))
                x_tiles.append(xt)

            for wname, n_big, dstT in (("q", 3, wqkvT), ("p", 1, wprojT)):
                wtiles = wtiles_all[wname]
                wsums = const.tile([128, n_big], dt.float32, name=f"wsums_{wname}")
                for g in range(n_big):
                    nc.vector.tensor_reduce(wsums[:, g:g + 1], wtiles[g][:],
                                            axis=AX.XY, op=ALU.add,
                                            apply_absolute_value=True)
                colsum = const.tile([128, 1], dt.float32, name=f"colsum_{wname}")
                nc.vector.tensor_reduce(colsum[:], wsums[:], axis=AX.X, op=ALU.add)
                tot_ps = w_ps.tile([1, 1], dt.float32, tag="tot")
                nc.tensor.matmul(tot_ps[:], colsum[:], ones_col[:],
                                 start=True, stop=True)
                meanc = const.tile([1, 1], dt.float32, name=f"meanc_{wname}")
                nc.vector.tensor_scalar(meanc[:], tot_ps[:],
                                        float(1.0 / (n_big * 768 * C)), EPS,
                                        op0=ALU.mult, op1=ALU.max)
                thr11 = const.tile([1, 1], dt.float32, name=f"thr11_{wname}")
                nc.vector.tensor_scalar(thr11[:], meanc[:], 0.5, None, op0=ALU.mult)
                thr_col = const.tile([128, 1], dt.float32, name=f"thrc_{wname}")
                nthr_col = const.tile([128, 1], dt.float32, name=f"nthrc_{wname}")
                inv_sw_col = const.tile([128, 1], dt.float32, name=f"iswc_{wname}")
                for src11, dst in ((thr11, thr_col), (meanc, inv_sw_col)):
                    ps = w_ps.tile([128, 1], dt.float32, tag="bc1")
                    nc.tensor.matmul(ps[:], ones_row[:], src11[:],
                                     start=True, stop=True)
                    nc.vector.tensor_copy(dst[:], ps[:])
                nc.vector.tensor_scalar(nthr_col[:], thr_col[:], -1.0, None,
                                        op0=ALU.mult)
                scales[wname] = (thr_col, nthr_col, inv_sw_col, meanc)

                for g in range(n_big):
                    w_t = wtiles[g]
                    for sch in range(NCC):
                        sub = w_t[:, sch, :]
                        bneg = wq_st.tile([128, C], dt.bfloat16, tag="bneg")
                        nc.gpsimd.tensor_scalar(bneg[:], sub, nthr_col[:], None,
                                                op0=ALU.is_le)
                        wq_t = wq_st.tile([128, C], dt.bfloat16, tag="wq")
                        nc.vector.scalar_tensor_tensor(wq_t[:], sub, thr_col[:],
                                                       bneg[:], op0=ALU.is_ge,
                                                       op1=ALU.subtract)
                        off = g * 768 + sch * 128
                        nc.sync.dma_start(dstT[:, :, off:off + 128], wq_t[:],
                                          transpose=True)

        swsq8 = const.tile([1, 1], dt.float32)
        nc.vector.tensor_scalar(swsq8[:], scales["q"][3][:], scales["q"][3][:],
                                0.125, op0=ALU.mult, op1=ALU.mult)

        # ---- phase X + M1 ------------------------------------------------
        xqT_pool = ctx.enter_context(tc.tile_pool(name="xqTall", bufs=1))
        xqT = xqT_pool.tile([128, NCC, N], dt.bfloat16)       # [p, cc, tok]
        xqTm = xqT_pool.tile([128, NCC, NQ], dt.bfloat16)
        v_p = ctx.enter_context(tc.tile_pool(name="v", bufs=1))
        qt_p = ctx.enter_context(tc.tile_pool(name="qt", bufs=1))
        vt = [v_p.tile([128, H * (D + 1)], dt.float32r, name=f"vt{t}")
              for t in range(TKV)]
        qt = [qt_p.tile([128, NQ], dt.float32r, name=f"qt{f}")
              for f in range(NCC)]
        alpha_bc = const.tile([128, NQ], dt.float32)

        with tc.tile_pool(name="xscratch", bufs=2) as xscratch, \
             tc.tile_pool(name="xdump", bufs=1) as xdump, \
             tc.tile_pool(name="xqst", bufs=3) as xqst, \
             tc.tile_pool(name="m1_ps", bufs=3, space="PSUM") as m1_ps, \
             tc.tile_pool(name="al_ps", bufs=1, space="PSUM") as al_ps:
            pools = {"scratch": xscratch, "dump": xdump, "stats": stats,
                     "xq": xqst, "eps_col": eps_col}

            # -- my-token chunks first (enables Q path early) --
            for tg in range(2):
                xm_big = x_tiles[tg]
                for tt in range(2):
                    t = tg * 2 + tt
                    xq = _quant_x_tile(nc, pools, xm_big[:, tt, :], gq_bc,
                                       inv_s_my[:, t:t + 1])
                    nc.sync.dma_start(xqTm[:, :, t * 128:(t + 1) * 128], xq[:],
                                        transpose=True)

            # alpha = inv_s_my * inv_sw^2/8, as a [128, NQ] broadcast tile
            tp = al_ps.tile([TQ, 128], dt.float32, tag="alT")
            nc.tensor.transpose(tp[:], inv_s_my[:], ident[:])
            al4 = xdump.tile([TQ, 128], dt.float32)
            nc.vector.tensor_copy(al4[:], tp[:])
            alrow = xdump.tile([1, NQ], dt.float32)
            for t in range(TQ):
                nc.sync.dma_start(alrow[:, t * 128:(t + 1) * 128], al4[t:t + 1, :])
            alrow2 = xdump.tile([1, NQ], dt.float32)
            nc.vector.tensor_scalar(alrow2[:], alrow[:], swsq8[:], None,
                                    op0=ALU.mult)
            ps = al_ps.tile([128, NQ], dt.float32, tag="alT")
            nc.tensor.matmul(ps[:], ones_row[:], alrow2[:], start=True, stop=True)
            nc.vector.tensor_copy(alpha_bc[:], ps[:])

            # Q^T [feat, tok] with fused alpha scale
            for f in range(NCC):
                ps = m1_ps.tile([128, NQ], dt.float32, tag="m1")
                for c in range(NCC):
                    nc.tensor.matmul(ps[:], wqkvT[:, c, f * 128:(f + 1) * 128],
                                     xqTm[:, c, :],
                                     start=(c == 0), stop=(c == NCC - 1))
                nc.vector.tensor_tensor(qt[f][:], ps[:], alpha_bc[:], op=ALU.mult)
                if debug_taps and f == 0:
                    nc.sync.dma_start(dbg["qt0"].ap(), qt[f][:].bitcast(dt.float32))

            # -- kv chunks, V matmuls interleaved --
            for tg in range(8):
                xb_big = x_tiles[2 + tg]
                for tt in range(2):
                    t = tg * 2 + tt
                    xq = _quant_x_tile(nc, pools, xb_big[:, tt, :], gq_bc,
                                       inv_s_all[:, t:t + 1])
                    if debug_taps and t == 0:
                        nc.sync.dma_start(dbg["xq0"].ap(), xq[:])
                    nc.sync.dma_start(xqT[:, :, t * 128:(t + 1) * 128], xq[:],
                                        transpose=True)
                    # per-chunk V scale column
                    vcol = stats.tile([128, 1], dt.float32, tag="vcol")
                    nc.vector.tensor_scalar(vcol[:], inv_s_all[:, t:t + 1],
                                            scales["q"][2][:], None, op0=ALU.mult)
                    v_re = vt[t][:].rearrange("p (h x) -> p h x", x=D + 1)
                    for half in range(2):
                        ps_full = m1_ps.tile([128, 512], dt.float32, tag="m1", name="psv")
                        ps = ps_full[:, 0:384]
                        for c in range(NCC):
                            nc.tensor.matmul(
                                ps[:], xqT[:, c, t * 128:(t + 1) * 128],
                                wqkvT[:, c, 2 * C + half * 384:2 * C + (half + 1) * 384],
                                start=(c == 0), stop=(c == NCC - 1))
                        nc.vector.tensor_scalar(
                            v_re[:, 6 * half:6 * half + 6, 0:D],
                            ps[:].rearrange("p (h x) -> p h x", x=D),
                            vcol[:], None, op0=ALU.mult)
                    nc.vector.memset(v_re[:, :, D:D + 1].bitcast(dt.float32), 1.0)
            if debug_taps:
                nc.sync.dma_start(dbg["vt0"].ap(), vt[0][:].bitcast(dt.float32))
                nc.sync.dma_start(dbg["inv_s_all"].ap(), inv_s_all[:])
                nc.sync.dma_start(dbg["inv_s_my"].ap(), inv_s_my[:])
                nc.sync.dma_start(dbg["alpha"].ap(), alpha_bc[:])
                nc.sync.dma_start(dbg["wqkvT0"].ap(), wqkvT[:, 0, :])

        # ---- phase A: attention (lazy K^T per feature chunk) -------------
        att_out = [attout_p.tile([128, C], dt.float32, name=f"ao{t}")
                   for t in range(TQ)]
        with tc.tile_pool(name="k_ps", bufs=1, space="PSUM") as k_ps, \
             tc.tile_pool(name="s_ps", bufs=2, space="PSUM") as s_ps, \
             tc.tile_pool(name="av_ps", bufs=1, space="PSUM") as av_ps, \
             tc.tile_pool(name="tp_ps", bufs=1, space="PSUM") as tp_ps, \
             tc.tile_pool(name="ktroll", bufs=2) as ktroll, \
             tc.tile_pool(name="aexp", bufs=2) as aexp, \
             tc.tile_pool(name="avsb", bufs=2) as avsb:
            def build_kt_block(ktf, f, t):
                ps = k_ps.tile([128, 512], dt.float32, tag="k", name="kps")
                for c in range(NCC):
                    nc.tensor.matmul(
                        ps[:], wqkvT[:, c, C + f * 128:C + (f + 1) * 128],
                        xqT[:, c, t * 512:(t + 1) * 512],
                        start=(c == 0), stop=(c == NCC - 1))
                nc.vector.tensor_copy(ktf[:, t * 512:(t + 1) * 512], ps[:])

            kt_cur = ktroll.tile([128, N], dt.float32r, tag="kt", name="kt0t")
            for t in range(4):
                build_kt_block(kt_cur, 0, t)
            if debug_taps:
                nc.sync.dma_start(dbg["kt0"].ap(), kt_cur[:].bitcast(dt.float32))
            def emit_qk_pair(sp, ktf, f, cch):
                # both heads of pair f share the kv chunk: head 2f into
                # columns 0:512, head 2f+1 into 512:1024 (separate banks)
                for hi, po in ((0, 0), (1, 64)):
                    nc.tensor.matmul(
                        sp[:, hi * NQ:(hi + 1) * NQ],
                        ktf[po:po + 64, cch * 128:(cch + 1) * 128],
                        qt[f][po:po + 64, :], start=True, stop=True)

            for f in range(NCC):
                ktf = kt_cur
                kt_next = None
                # one exp call covers BOTH heads of the pair (same kv chunk
                # -> same per-partition scale): halves ACT call count.
                av0 = av_ps.tile([D + 1, NQ], dt.float32, tag="av0")
                av1 = av_ps.tile([D + 1, NQ], dt.float32, tag="av1")
                sps = [s_ps.tile([128, 2 * NQ], dt.float32, tag="s", name="sp0")]
                emit_qk_pair(sps[0], ktf, f, 0)
                for cch in range(TKV):
                    if cch + 1 < TKV:
                        sp1 = s_ps.tile([128, 2 * NQ], dt.float32, tag="s",
                                        name="sp1")
                        emit_qk_pair(sp1, ktf, f, cch + 1)
                        sps.append(sp1)
                    if f + 1 < NCC and cch % 4 == 1:
                        if kt_next is None:
                            kt_next = ktroll.tile([128, N], dt.float32r,
                                                  tag="kt", name="ktn")
                        build_kt_block(kt_next, f + 1, cch // 4)
                    ae = aexp.tile([128, 2 * NQ], dt.float32r, tag="ae")
                    nc.scalar.activation(ae[:], sps[cch][:], AF.Exp,
                                         scale=inv_s_all[:, cch:cch + 1])
                    for hi, av in ((0, av0), (1, av1)):
                        h = 2 * f + hi
                        nc.tensor.matmul(
                            av[:], vt[cch][:, h * (D + 1):(h + 1) * (D + 1)],
                            ae[:, hi * NQ:(hi + 1) * NQ],
                            start=(cch == 0), stop=(cch == TKV - 1))
                for hi, av in ((0, av0), (1, av1)):
                    h = 2 * f + hi
                    av_sb = avsb.tile([D + 1, NQ], dt.float32, tag="avsb")
                    nc.vector.tensor_copy(av_sb[:], av[:])
                    if debug_taps and h == 0:
                        nc.sync.dma_start(dbg["avsb0"].ap(), av_sb[:])
                    for t in range(TQ):
                        tp = tp_ps.tile([128, D + 1], dt.float32, tag="tp")
                        nc.tensor.transpose(tp[:], av_sb[:, t * 128:(t + 1) * 128],
                                            ident[0:D + 1, 0:D + 1])
                        dcol = stats.tile([128, 1], dt.float32, tag="dcol")
                        nc.vector.reciprocal(dcol[:], tp[:, D:D + 1])
                        nc.vector.tensor_scalar(att_out[t][:, h * D:(h + 1) * D],
                                                tp[:, 0:D], dcol[:], None,
                                                op0=ALU.mult)
                kt_cur = kt_next

        # ---- phase P: proj bitlinear -------------------------------------
        with tc.tile_pool(name="p_scr", bufs=1) as p_scr, \
             tc.tile_pool(name="p_dump", bufs=1) as p_dump, \
             tc.tile_pool(name="p_xq", bufs=2) as p_xq, \
             tc.tile_pool(name="xq2T", bufs=1) as xq2T_p, \
             tc.tile_pool(name="m2_ps", bufs=3, space="PSUM") as m2_ps, \
             tc.tile_pool(name="outsb", bufs=2) as outsb:
            xq2T = xq2T_p.tile([128, NCC, NQ], dt.bfloat16)
            pools2 = {"scratch": p_scr, "dump": p_dump, "stats": stats,
                      "xq": p_xq, "eps_col": eps_col}
            inv_s2 = const.tile([128, TQ], dt.float32)
            if debug_taps:
                nc.sync.dma_start(dbg["att0"].ap(), att_out[0][:])
            for t in range(TQ):
                xq2 = _quant_x_tile(nc, pools2, att_out[t][:], gp_bc,
                                    inv_s2[:, t:t + 1])
                nc.sync.dma_start(xq2T[:, :, t * 128:(t + 1) * 128], xq2[:],
                                      transpose=True)
            pcol = const.tile([128, TQ], dt.float32)
            nc.vector.tensor_scalar(pcol[:], inv_s2[:], scales["p"][2][:],
                                    None, op0=ALU.mult)
            for t in range(TQ):
                o_sb = outsb.tile([128, C], dt.float32, tag="osb")
                for half in range(2):
                    ps = m2_ps.tile([128, 384], dt.float32, tag="m2")
                    for c in range(NCC):
                        nc.tensor.matmul(
                            ps[:], xq2T[:, c, t * 128:(t + 1) * 128],
                            wprojT[:, c, half * 384:(half + 1) * 384],
                            start=(c == 0), stop=(c == NCC - 1))
                    nc.scalar.mul(o_sb[:, half * 384:(half + 1) * 384],
                                  ps[:], pcol[:, t:t + 1])
                nc.sync.dma_start(out_d.ap()[t * 128:(t + 1) * 128, :], o_sb[:])

    nc.compile()
    return nc


def _get_program(debug_taps=False):
    key = ("nc", debug_taps)
    if key not in _CACHE:
        _CACHE[key] = build_program(debug_taps)
    return _CACHE[key]


def kernel(x, w_qkv, g_qkv, w_proj, g_proj, _trace=False, _debug_taps=False,
           **trace_kwargs):
    x = np.ascontiguousarray(np.asarray(x, dtype=np.float32))
    w_qkv = np.ascontiguousarray(np.asarray(w_qkv, dtype=np.float32))
    w_proj = np.ascontiguousarray(np.asarray(w_proj, dtype=np.float32))
    gq = np.ascontiguousarray(np.asarray(g_qkv, dtype=np.float32).reshape(1, C))
    gp = np.ascontiguousarray(np.asarray(g_proj, dtype=np.float32).reshape(1, C))

    nc = _get_program(_debug_taps)
    in_maps = []
    for core in range(8):
        b, j = core // 4, core % 4
        in_maps.append({
            "xb": x[b],
            "xm": x[b, j * NQ:(j + 1) * NQ],
            "wqkv": w_qkv,
            "wproj": w_proj,
            "gq": gq,
            "gp": gp,
        })
    res = run_bass_kernel_spmd(nc, in_maps, list(range(8)), trace=_trace,
                               **trace_kwargs)
    out = np.empty((B, N, C), dtype=np.float32)
    for core in range(8):
        b, j = core // 4, core % 4
        out[b, j * NQ:(j + 1) * NQ] = res.results[core]["out"]
    if _trace or _debug_taps:
        return out, res
    return out
